# revision 5
# baseline (speedup 1.0000x reference)
"""GAT layer (nn_GATLayerAdj) Trainium2 Bass kernel, 8-core SPMD — v3.

Reference computation (N=1024, di=do=64):
    a[i,j]  = x[j]@w_src + x[i]@w_tgt + bw        (attention logits)
    att     = softmax_j(where(adj>0, a, -1e16))
    y[i,j,:]= relu(x[j]@WfS.T + x[i]@WfT.T + bf)
    o[i,:]  = sum_j att[i,j] * y[i,j,:]

Sharding: target-node dim i split across 8 cores (128 rows each).

Algebraic restructurings (vs the ~74us v1):
  1. Row-softmax is invariant to the per-row shift a_tgt[i]+bw, so the
     attention weights reduce to eT[j,i] = exp(a_src[j]) * adjT[j,i] —
     one per-partition ACT scale-copy per j-chunk, directly in
     TRANSPOSED layout (j on partitions): no logits outer product, no
     big exp, no PE transposes.
  2. relu(ys_j + u_i) = max(ys_j, -u_i) + u_i. Most of the N^2*do/8
     elementwise volume is ONE DVE tensor_tensor max per half-chunk
     (bf16 2x mode); v1 needed an add + a relu. The +u_i correction is
     applied at evacuation: o = t_acc*(1/s) - nurep2*(sM/s), where
     nurep2 is a diagonal-replicated -u tile (4 partition-broadcast
     DMAs) and sM is the partial attention row-sum over max-form
     chunks (mask-weighted reduce of per-chunk row sums).
  3. A few half-chunks run in relu-form on otherwise-idle engines:
     Pool computes z = ys - (-u) (TensorTensor subtract is the one
     elementwise op its Q7 ucode supports; max is not), ACT evacuates
     relu(z). Those (chunk, half)s are excluded from sM via a 0/1 mask
     built with memsets, so the evacuation correction stays exact.

Engine budget per core (~measured): DVE 13 maxes ~30us (the wall),
Pool 3 subtracts ~25us, ACT relus+small ops ~19us, PE reduce
(4x4 col-tiled accumulating matmuls, tile_position groups run
concurrently) ~24us of slices at ~3x overlap.

Numerics: bf16 inputs, fp32 PSUM accumulation; max keeps one operand
exact, u enters through the same bf16 rounding as v1.
"""

from contextlib import ExitStack

import numpy as np
import ml_dtypes

import concourse.bass as bass
import concourse.tile as tile
from concourse import bacc, mybir
from concourse.bass_utils import run_bass_kernel_spmd

# Lighter TileContext exit: stock emits drain + full butterfly barrier +
# sem clears + second butterfly (~11us). Engines already sync at program
# end; keep the drain (output DMA completion), a sem-only rendezvous
# before the clears, and drop the trailing barrier.
import concourse.tile as _tile_mod

if not getattr(_tile_mod, "_exit_trimmed", False):
    def _drain_and_barrier_trim(self, tick_clock, wait_clock):
        from concourse.tile import ScopedClock
        nc = self.nc
        drain_inst = nc.sync.drain()
        wait_clock.add_sem_waits(
            drain_inst.ins, ScopedClock({None: tick_clock.global_clock})
        )
        # parallel rendezvous: every engine incs one sem; gpsimd waits,
        # clears the tile sems, and the program ends (engines sync at
        # program completion anyway - no trailing butterfly needed)
        exit_sem = nc.alloc_semaphore("exit_rdv")
        for eng in (nc.sync, nc.tensor, nc.vector, nc.scalar):
            eng.nop(nofuse=True).then_inc(exit_sem, 1)
        nc.gpsimd.wait_ge(exit_sem, 4)
        assert self.sems is not None
        popped = nc._tile_sem_poison_stack.pop()
        assert popped is self._sem_poison
        nc.clear_and_free_semaphores(list(self.sems.allocated().values()))
        nc.gpsimd.sem_clear(range(exit_sem.num, exit_sem.num + 1))

    _tile_mod.TileContext._drain_and_barrier = _drain_and_barrier_trim
    _tile_mod._exit_trimmed = True

N = 1024
DI = 64
DO = 64
N_CORES = 8
ROWS = N // N_CORES          # 128 target rows per core
NCHUNK = N // 128            # 8 j-chunks
F_FULL = ROWS * DO           # 8192 free size of (i, d)
HALF = F_FULL // 2           # 4096: half-chunk unit
QUART = F_FULL // 4          # 2048

f32 = mybir.dt.float32
bf16 = mybir.dt.bfloat16
AF = mybir.ActivationFunctionType
ALU = mybir.AluOpType
AX = mybir.AxisListType

# (chunk, half) pairs computed in relu-form: Pool subtract + ACT relu.
# All others are DVE max-form. Chunk 0 and 7 must stay DVE (they open /
# close the PE accumulation chains promptly).
POOL_R_HALVES = frozenset({(1, 1), (4, 0), (6, 1)})

_CACHE = {}


def _build_program():
    nc = bacc.Bacc("TRN2", target_bir_lowering=False, debug=False,
                   num_devices=N_CORES)

    # ---- DRAM I/O (concatenated to cut ~600ns-per-trigger DMA issue) ----
    # xw = [xbTa | nwfta]  [65, 192], xm = [xT | wfsT | ws]  [64, 1089]
    xw_d = nc.dram_tensor("xw", [DI + 1, ROWS + DO], bf16,
                          kind="ExternalInput").ap()
    xm_d = nc.dram_tensor("xm", [DI, N + DO + 1], bf16,
                          kind="ExternalInput").ap()
    adjT_d = nc.dram_tensor("adjT", [ROWS, N], bf16, kind="ExternalInput").ap()
    o_d = nc.dram_tensor("o", [128, 2048], f32, kind="ExternalOutput").ap()

    with tile.TileContext(nc) as tc, ExitStack() as ctx:
        cons = ctx.enter_context(tc.tile_pool(name="cons", bufs=1))
        rp = ctx.enter_context(tc.tile_pool(name="rp", bufs=3))
        zp = ctx.enter_context(tc.tile_pool(name="zp", bufs=2))
        psp = ctx.enter_context(tc.tile_pool(name="psp", bufs=3, space="PSUM"))
        accs = ctx.enter_context(tc.tile_pool(name="accs", bufs=1, space="PSUM"))
        accp = ctx.enter_context(tc.tile_pool(name="accp", bufs=1, space="PSUM"))

        # ---- input DMAs: u-chain tensor first ----
        xw_t = cons.tile([DI + 1, ROWS + DO], bf16)
        nc.sync.dma_start(xw_t[:], xw_d[:, :])
        xm_t = cons.tile([DI, N + DO + 1], bf16)
        nc.sync.dma_start(xm_t[:], xm_d[:, :])
        xbTa_t = xw_t[:, :ROWS]
        nwfta_t = xw_t[:, ROWS:ROWS + DO]
        xT_t = xm_t[:, :N]
        wfsT_t = xm_t[:, N:N + DO]
        ws_t = xm_t[:, N + DO:N + DO + 1]
        adjT_t = cons.tile([ROWS, N], bf16)
        nc.gpsimd.dma_start(adjT_t[:], adjT_d[:, :])

        # ---- nu = -(xb@WfT.T + bf)  [128, 64] (K=65 ones-row trick) ----
        nu_ps = psp.tile([ROWS, DO], f32, tag="pre")
        nc.tensor.matmul(nu_ps[:], xbTa_t, nwfta_t, start=True, stop=True)
        nu_sb = cons.tile([ROWS, DO], bf16)
        nc.scalar.copy(nu_sb[:], nu_ps[:])
        # stage flat to DRAM, then partition-broadcast reads (4 quarters
        # split over both HWDGE queues so the first maxes start sooner)
        nu_dram = nc.dram_tensor("nu_stage", [F_FULL], bf16).ap()
        nc.sync.dma_start(out=nu_dram.rearrange("(i d) -> i d", i=ROWS),
                          in_=nu_sb[:, :])
        nurep = cons.tile([128, F_FULL], bf16)
        for q, eng in ((0, nc.sync), (1, nc.scalar), (2, nc.sync),
                       (3, nc.scalar)):
            sl = slice(QUART * q, QUART * (q + 1))
            src = nu_dram[sl]
            bsrc = bass.AP(tensor=src.tensor, offset=src.offset,
                           ap=[[0, 128]] + [list(d) for d in src.ap])
            eng.dma_start(out=nurep[:, sl], in_=bsrc)
        # diagonal-replicated -u for the evacuation fix-up:
        # nurep2[p, f] = nu[32*(p//32) + f//64, f%64]
        nurep2 = cons.tile([128, 2048], bf16)
        for b in range(4):
            src = nu_dram[2048 * b:2048 * (b + 1)]
            bsrc = bass.AP(tensor=src.tensor, offset=src.offset,
                           ap=[[0, 32]] + [list(d) for d in src.ap])
            nc.gpsimd.dma_start(out=nurep2[32 * b:32 * (b + 1), :], in_=bsrc)

        # ---- a_src row + exp -> es, re-laid out per-partition ----
        es_row = cons.tile([1, N], bf16)
        for h in range(2):
            hs = slice(512 * h, 512 * (h + 1))
            asp = psp.tile([1, 512], f32, tag="pre", name=f"asp{h}")
            nc.tensor.matmul(asp[:], ws_t, xT_t[:, hs], start=True, stop=True)
            nc.scalar.activation(es_row[:, hs], asp[:], AF.Exp)
        es_dram = nc.dram_tensor("es_stage", [N], bf16).ap()
        nc.gpsimd.dma_start(out=es_dram.rearrange("(o f) -> o f", o=1),
                            in_=es_row[:, :])
        # f32: ACT scale APs must be FP32; the gpsimd (SWDGE) DMA casts
        es_col = cons.tile([128, NCHUNK], f32)
        nc.gpsimd.dma_start(out=es_col[:, :],
                            in_=es_dram.rearrange("(c p) -> p c", p=128))

        # ---- ys chunks: ys_jp[j_local, 64*c + d] = ys[128*c + j_local, d] ----
        ys_jp = cons.tile([128, NCHUNK * DO], bf16)
        for c in range(NCHUNK):
            ysp = psp.tile([128, DO], f32, tag="pre", name=f"ysp{c}")
            nc.tensor.matmul(ysp[:], xT_t[:, 128 * c:128 * (c + 1)], wfsT_t,
                             start=True, stop=True)
            nc.scalar.copy(ys_jp[:, DO * c:DO * (c + 1)], ysp[:])

        # ---- eT chunks (ACT scale-copy) + per-chunk row sums ----
        onescol = cons.tile([128, 1], bf16)
        nc.vector.memset(onescol[:], 1.0)
        # maskM[p, c] = 1 iff (c, p//64) is max-form (in the sM sum)
        maskM = cons.tile([128, NCHUNK], f32)
        nc.gpsimd.memset(maskM[:], 1.0)
        for (c, h) in sorted(POOL_R_HALVES):
            nc.gpsimd.memset(maskM[64 * h:64 * (h + 1), c:c + 1], 0.0)
        et_all = cons.tile([128, N], bf16)
        rsum_ps = accs.tile([ROWS, NCHUNK], f32, tag="acc")
        for c in range(NCHUNK):
            cs = slice(128 * c, 128 * (c + 1))
            nc.scalar.activation(et_all[:, cs], adjT_t[:, cs], AF.Copy,
                                 bias=0.0, scale=es_col[:, c:c + 1])
            nc.tensor.matmul(rsum_ps[:, c:c + 1], et_all[:, cs], onescol[:],
                             start=True, stop=True, skip_group_check=True)
        s_t = cons.tile([ROWS, 1], f32)
        sM_t = cons.tile([ROWS, 1], f32)
        junk8 = cons.tile([ROWS, NCHUNK], f32)
        r_t = cons.tile([ROWS, 1], f32)
        qcol = cons.tile([ROWS, 1], f32)

        # ---- max/relu build + reduce, software-pipelined ----
        SKEW = 2
        t_acc = accp.tile([128, 2048], f32, tag="acc")
        r_tiles = {}

        def emit_build(c):
            r_c = rp.tile([128, F_FULL], bf16, name="r_c")
            r_tiles[c] = r_c
            ys_c = ys_jp[:, DO * c:DO * (c + 1)]
            ys_b = ys_c.rearrange("p d -> p () d").broadcast_to(
                (128, HALF // DO, DO))
            for h in range(2):
                sl = slice(HALF * h, HALF * (h + 1))
                rv = r_c[:, sl].rearrange("p (i d) -> p i d", i=HALF // DO)
                nuv = nurep[:, sl].rearrange("p (i d) -> p i d", i=HALF // DO)
                if (c, h) in POOL_R_HALVES:
                    # relu-form: z = ys - (-u) on Pool, relu on ACT
                    z = zp.tile([128, HALF], bf16, name="z")
                    zv = z[:, :].rearrange("p (i d) -> p i d", i=HALF // DO)
                    nc.gpsimd.tensor_tensor(zv, ys_b, nuv, ALU.subtract)
                    nc.scalar.activation(r_c[:, sl], z[:], AF.Relu)
                else:
                    nc.vector.tensor_tensor(rv, ys_b, nuv, ALU.max)

        def emit_reduce(c):
            r_c = r_tiles.pop(c)
            for n2 in range(4):
                for b in range(4):
                    nc.tensor.matmul(
                        t_acc[32 * b:32 * (b + 1), 512 * n2:512 * (n2 + 1)],
                        et_all[:, 128 * c + 32 * b:128 * c + 32 * (b + 1)],
                        r_c[:, 2048 * b + 512 * n2:2048 * b + 512 * (n2 + 1)],
                        start=(c == 0),
                        stop=(c == NCHUNK - 1),
                        skip_group_check=True,
                        tile_position=(0, 32 * b),
                    )

        for cc in range(NCHUNK + SKEW):
            if cc < NCHUNK:
                emit_build(cc)
            if cc == 3:
                # mid-DVE-queue: rsum_ps is long done by now
                nc.vector.tensor_reduce(s_t[:], rsum_ps[:], AX.X, ALU.add)
                nc.vector.scalar_tensor_tensor(
                    junk8[:], rsum_ps[:], 0.0, maskM[:],
                    ALU.bypass, ALU.mult, accum_out=sM_t[:])
                nc.vector.reciprocal(r_t[:], s_t[:])
                nc.vector.tensor_tensor(qcol[:], sM_t[:], r_t[:], ALU.mult)
            if cc >= SKEW:
                emit_reduce(cc - SKEW)

        # scaled fix-up tile: nurep2q = nurep2 * (sM/s)  (ACT, has slack)
        nurep2q = cons.tile([128, 2048], bf16)
        nc.scalar.activation(nurep2q[:], nurep2[:], AF.Copy,
                             bias=0.0, scale=qcol[:])

        # ---- tail: o = t_acc*(1/s) - nurep2q, fused on DVE; DMA out ----
        t_sb = cons.tile([128, 2048], f32)
        for n2 in range(4):
            sl = slice(512 * n2, 512 * (n2 + 1))
            nc.vector.scalar_tensor_tensor(
                t_sb[:, sl], t_acc[:, sl], r_t[:], nurep2q[:, sl],
                ALU.mult, ALU.subtract)
            eng = nc.sync if n2 % 2 == 0 else nc.scalar
            eng.dma_start(o_d[:, sl], t_sb[:, sl])

    nc.compile()
    return nc


def _prep_inputs(x, adj, Wf, bf_, Ww, bw):
    b = ml_dtypes.bfloat16
    xT = np.ascontiguousarray(x.T)                        # [64, N]
    wfsT = np.ascontiguousarray(Wf[:, :DI].T)             # [64, 64]
    ws = Ww[0, :DI].reshape(DI, 1)                        # [64, 1]
    xm = np.hstack([xT, wfsT, ws]).astype(b)              # [64, 1089]
    nwfta = -np.vstack([Wf[:, DI:].T, bf_[None, :]])      # [65, 64]

    in_maps = []
    for c in range(N_CORES):
        blk = slice(ROWS * c, ROWS * (c + 1))
        xbTa = np.vstack([x[blk].T, np.ones((1, ROWS), np.float32)])
        xw = np.hstack([xbTa, nwfta]).astype(b)           # [65, 192]
        # adjT chunk-major: adjT[j_loc, 128c + i] = adj[blk0+i, 128c+j_loc]
        adjT = (adj[blk].T.reshape(NCHUNK, 128, ROWS)
                .transpose(1, 0, 2).reshape(128, N))
        in_maps.append(dict(xm=xm, xw=xw,
                            adjT=np.ascontiguousarray(adjT).astype(b)))
    return in_maps


def get_program():
    if "nc" not in _CACHE:
        _CACHE["nc"] = _build_program()
    return _CACHE["nc"]


def kernel(x, adj, Wf, bf, Ww, bw):
    x = np.asarray(x, dtype=np.float32)
    adj = np.asarray(adj, dtype=np.int32)
    Wf = np.asarray(Wf, dtype=np.float32)
    bf_ = np.asarray(bf, dtype=np.float32)
    Ww = np.asarray(Ww, dtype=np.float32)
    bw = np.asarray(bw, dtype=np.float32)
    assert x.shape == (N, DI) and adj.shape == (N, N)

    nc = get_program()
    in_maps = _prep_inputs(x, adj, Wf, bf_, Ww, bw)
    res = run_bass_kernel_spmd(nc, in_maps, core_ids=list(range(N_CORES)))
    p_idx = np.arange(128)
    col0 = (p_idx % 32) * DO
    out = np.empty((N, DO), np.float32)
    for c in range(N_CORES):
        t = res.results[c]["o"]                      # [128, 2048]
        out[ROWS * c:ROWS * (c + 1)] = t[p_idx[:, None],
                                         col0[:, None] + np.arange(DO)[None, :]]
    return out


# revision 9
# speedup vs baseline: 1.2222x; 1.2222x over previous
"""GAT layer (nn_GATLayerAdj) Trainium2 Bass kernel, 8-core SPMD — v3.

Reference computation (N=1024, di=do=64):
    a[i,j]  = x[j]@w_src + x[i]@w_tgt + bw        (attention logits)
    att     = softmax_j(where(adj>0, a, -1e16))
    y[i,j,:]= relu(x[j]@WfS.T + x[i]@WfT.T + bf)
    o[i,:]  = sum_j att[i,j] * y[i,j,:]

Sharding: target-node dim i split across 8 cores (128 rows each).

Algebraic restructurings (vs the ~74us v1):
  1. Row-softmax is invariant to the per-row shift a_tgt[i]+bw, so the
     attention weights reduce to eT[j,i] = exp(a_src[j]) * adjT[j,i] —
     one per-partition ACT scale-copy per j-chunk, directly in
     TRANSPOSED layout (j on partitions): no logits outer product, no
     big exp, no PE transposes.
  2. relu(ys_j + u_i) = max(ys_j, -u_i) + u_i. Most of the N^2*do/8
     elementwise volume is ONE DVE tensor_tensor max per half-chunk
     (bf16 2x mode); v1 needed an add + a relu. The +u_i correction is
     applied at evacuation: o = t_acc*(1/s) - nurep2*(sM/s), where
     nurep2 is a diagonal-replicated -u tile (4 partition-broadcast
     DMAs) and sM is the partial attention row-sum over max-form
     chunks (mask-weighted reduce of per-chunk row sums).
  3. A few half-chunks run in relu-form on otherwise-idle engines:
     Pool computes z = ys - (-u) (TensorTensor subtract is the one
     elementwise op its Q7 ucode supports; max is not), ACT evacuates
     relu(z). Those (chunk, half)s are excluded from sM via a 0/1 mask
     built with memsets, so the evacuation correction stays exact.

Engine budget per core (~measured): DVE 13 maxes ~30us (the wall),
Pool 3 subtracts ~25us, ACT relus+small ops ~19us, PE reduce
(4x4 col-tiled accumulating matmuls, tile_position groups run
concurrently) ~24us of slices at ~3x overlap.

Numerics: bf16 inputs, fp32 PSUM accumulation; max keeps one operand
exact, u enters through the same bf16 rounding as v1.
"""

from contextlib import ExitStack

import numpy as np
import ml_dtypes

import concourse.bass as bass
import concourse.tile as tile
from concourse import bacc, mybir
from concourse.bass_utils import run_bass_kernel_spmd

# Lighter TileContext exit: stock emits drain + full butterfly barrier +
# sem clears + second butterfly (~11us). Engines already sync at program
# end; keep the drain (output DMA completion), a sem-only rendezvous
# before the clears, and drop the trailing barrier.
import concourse.tile as _tile_mod

if not getattr(_tile_mod, "_exit_trimmed", False):
    def _drain_and_barrier_trim(self, tick_clock, wait_clock):
        from concourse.tile import ScopedClock
        nc = self.nc
        drain_inst = nc.sync.drain()
        wait_clock.add_sem_waits(
            drain_inst.ins, ScopedClock({None: tick_clock.global_clock})
        )
        # parallel rendezvous: every engine incs one sem; gpsimd waits,
        # clears the tile sems, and the program ends (engines sync at
        # program completion anyway - no trailing butterfly needed)
        exit_sem = nc.alloc_semaphore("exit_rdv")
        for eng in (nc.sync, nc.tensor, nc.vector, nc.scalar):
            eng.nop(nofuse=True).then_inc(exit_sem, 1)
        nc.gpsimd.wait_ge(exit_sem, 4)
        assert self.sems is not None
        popped = nc._tile_sem_poison_stack.pop()
        assert popped is self._sem_poison
        nc.clear_and_free_semaphores(list(self.sems.allocated().values()))
        nc.gpsimd.sem_clear(range(exit_sem.num, exit_sem.num + 1))

    _tile_mod.TileContext._drain_and_barrier = _drain_and_barrier_trim
    _tile_mod._exit_trimmed = True

N = 1024
DI = 64
DO = 64
N_CORES = 8
ROWS = N // N_CORES          # 128 target rows per core
NCHUNK = N // 128            # 8 j-chunks
F_FULL = ROWS * DO           # 8192 free size of (i, d)
HALF = F_FULL // 2           # 4096: half-chunk unit
QUART = F_FULL // 4          # 2048

f32 = mybir.dt.float32
bf16 = mybir.dt.bfloat16
AF = mybir.ActivationFunctionType
ALU = mybir.AluOpType
AX = mybir.AxisListType

# (chunk, half) pairs computed in relu-form: Pool subtract + ACT relu.
# Measured: Pool TensorTensor is ~10us per half AND its SBUF traffic
# slows concurrent DVE maxes ~4x — strictly a loss. Keep empty.
POOL_R_HALVES = frozenset()

_CACHE = {}


def _build_program():
    nc = bacc.Bacc("TRN2", target_bir_lowering=False, debug=False,
                   num_devices=N_CORES)

    # ---- DRAM I/O (concatenated to cut ~600ns-per-trigger DMA issue) ----
    # xw = [xbTa | nwfta]  [65, 192], xm = [xT | wfsT | ws]  [64, 1089]
    xw_d = nc.dram_tensor("xw", [DI + 1, ROWS + DO], bf16,
                          kind="ExternalInput").ap()
    xm_d = nc.dram_tensor("xm", [DI, N + DO + 1], bf16,
                          kind="ExternalInput").ap()
    adjT_d = nc.dram_tensor("adjT", [ROWS, N], bf16, kind="ExternalInput").ap()
    o_d = nc.dram_tensor("o", [128, 2048], f32, kind="ExternalOutput").ap()

    with tile.TileContext(nc) as tc, ExitStack() as ctx:
        cons = ctx.enter_context(tc.tile_pool(name="cons", bufs=1))
        rp = ctx.enter_context(tc.tile_pool(name="rp", bufs=3))
        zp = ctx.enter_context(tc.tile_pool(name="zp", bufs=2))
        psp = ctx.enter_context(tc.tile_pool(name="psp", bufs=3, space="PSUM"))
        accs = ctx.enter_context(tc.tile_pool(name="accs", bufs=1, space="PSUM"))
        accp = ctx.enter_context(tc.tile_pool(name="accp", bufs=1, space="PSUM"))

        # ---- input DMAs: u-chain tensor first ----
        xw_t = cons.tile([DI + 1, ROWS + DO], bf16)
        nc.sync.dma_start(xw_t[:], xw_d[:, :])
        xm_t = cons.tile([DI, N + DO + 1], bf16)
        nc.sync.dma_start(xm_t[:], xm_d[:, :])
        xbTa_t = xw_t[:, :ROWS]
        nwfta_t = xw_t[:, ROWS:ROWS + DO]
        xT_t = xm_t[:, :N]
        wfsT_t = xm_t[:, N:N + DO]
        ws_t = xm_t[:, N + DO:N + DO + 1]
        adjT_t = cons.tile([ROWS, N], bf16)
        nc.gpsimd.dma_start(adjT_t[:], adjT_d[:, :])

        # ---- nu = -(xb@WfT.T + bf)  [128, 64] (K=65 ones-row trick) ----
        # The whole chain gates the DVE maxes (the critical engine), so
        # it runs at scheduler priority 0.
        nurep = cons.tile([128, F_FULL], bf16)
        nu_dram = nc.dram_tensor("nu_stage", [F_FULL], bf16).ap()
        nu_sb = cons.tile([ROWS, DO], bf16)
        with tc.high_priority():
            nu_ps = psp.tile([ROWS, DO], f32, tag="pre")
            nc.tensor.matmul(nu_ps[:], xbTa_t, nwfta_t, start=True, stop=True)
            nc.scalar.copy(nu_sb[:], nu_ps[:])
            # stage flat to DRAM, then partition-broadcast reads (4
            # quarters split over both HWDGE queues)
            nc.sync.dma_start(out=nu_dram.rearrange("(i d) -> i d", i=ROWS),
                              in_=nu_sb[:, :])
            for q, eng in ((0, nc.sync), (1, nc.scalar), (2, nc.sync),
                           (3, nc.scalar)):
                sl = slice(QUART * q, QUART * (q + 1))
                src = nu_dram[sl]
                bsrc = bass.AP(tensor=src.tensor, offset=src.offset,
                               ap=[[0, 128]] + [list(d) for d in src.ap])
                eng.dma_start(out=nurep[:, sl], in_=bsrc)
        # diagonal-replicated -u for the evacuation fix-up:
        # nurep2[p, f] = nu[32*(p//32) + f//64, f%64]
        nurep2 = cons.tile([128, 2048], bf16)
        for b in range(4):
            src = nu_dram[2048 * b:2048 * (b + 1)]
            bsrc = bass.AP(tensor=src.tensor, offset=src.offset,
                           ap=[[0, 32]] + [list(d) for d in src.ap])
            nc.gpsimd.dma_start(out=nurep2[32 * b:32 * (b + 1), :], in_=bsrc)

        # ---- a_src row + exp -> es, re-laid out per-partition ----
        es_row = cons.tile([1, N], bf16)
        for h in range(2):
            hs = slice(512 * h, 512 * (h + 1))
            asp = psp.tile([1, 512], f32, tag="pre", name=f"asp{h}")
            nc.tensor.matmul(asp[:], ws_t, xT_t[:, hs], start=True, stop=True)
            nc.scalar.activation(es_row[:, hs], asp[:], AF.Exp)
        es_dram = nc.dram_tensor("es_stage", [N], bf16).ap()
        nc.gpsimd.dma_start(out=es_dram.rearrange("(o f) -> o f", o=1),
                            in_=es_row[:, :])
        # f32: ACT scale APs must be FP32; the gpsimd (SWDGE) DMA casts
        es_col = cons.tile([128, NCHUNK], f32)
        nc.gpsimd.dma_start(out=es_col[:, :],
                            in_=es_dram.rearrange("(c p) -> p c", p=128))

        # ---- ys chunks: ys_jp[j_local, 64*c + d] = ys[128*c + j_local, d] ----
        ys_jp = cons.tile([128, NCHUNK * DO], bf16)
        for c in range(NCHUNK):
            ysp = psp.tile([128, DO], f32, tag="pre", name=f"ysp{c}")
            nc.tensor.matmul(ysp[:], xT_t[:, 128 * c:128 * (c + 1)], wfsT_t,
                             start=True, stop=True)
            nc.scalar.copy(ys_jp[:, DO * c:DO * (c + 1)], ysp[:])

        # ---- eT chunks (ACT scale-copy) + per-chunk row sums ----
        onescol = cons.tile([128, 1], bf16)
        nc.vector.memset(onescol[:], 1.0)
        mixed = bool(POOL_R_HALVES)
        if mixed:
            # maskM[p, c] = 1 iff (c, p//64) is max-form (in the sM sum)
            maskM = cons.tile([128, NCHUNK], f32)
            nc.gpsimd.memset(maskM[:], 1.0)
            for (c, h) in sorted(POOL_R_HALVES):
                nc.gpsimd.memset(maskM[64 * h:64 * (h + 1), c:c + 1], 0.0)
            sM_t = cons.tile([ROWS, 1], f32)
            junk8 = cons.tile([ROWS, NCHUNK], f32)
            qcol = cons.tile([ROWS, 1], f32)
        et_all = cons.tile([128, N], bf16)
        rsum_ps = accs.tile([ROWS, NCHUNK], f32, tag="acc")
        for c in range(NCHUNK):
            cs = slice(128 * c, 128 * (c + 1))
            nc.scalar.activation(et_all[:, cs], adjT_t[:, cs], AF.Copy,
                                 bias=0.0, scale=es_col[:, c:c + 1])
            nc.tensor.matmul(rsum_ps[:, c:c + 1], et_all[:, cs], onescol[:],
                             start=True, stop=True, skip_group_check=True)
        s_t = cons.tile([ROWS, 1], f32)
        r_t = cons.tile([ROWS, 1], f32)

        # ---- max/relu build + reduce, software-pipelined ----
        SKEW = 2
        t_acc = accp.tile([128, 2048], f32, tag="acc")
        r_tiles = {}

        def emit_build(c):
            r_c = rp.tile([128, F_FULL], bf16, name="r_c")
            r_tiles[c] = r_c
            ys_c = ys_jp[:, DO * c:DO * (c + 1)]
            ys_b = ys_c.rearrange("p d -> p () d").broadcast_to(
                (128, HALF // DO, DO))
            for h in range(2):
                sl = slice(HALF * h, HALF * (h + 1))
                rv = r_c[:, sl].rearrange("p (i d) -> p i d", i=HALF // DO)
                nuv = nurep[:, sl].rearrange("p (i d) -> p i d", i=HALF // DO)
                if (c, h) in POOL_R_HALVES:
                    # relu-form: z = ys - (-u) on Pool, relu on ACT
                    z = zp.tile([128, HALF], bf16, name="z")
                    zv = z[:, :].rearrange("p (i d) -> p i d", i=HALF // DO)
                    nc.gpsimd.tensor_tensor(zv, ys_b, nuv, ALU.subtract)
                    nc.scalar.activation(r_c[:, sl], z[:], AF.Relu)
                else:
                    nc.vector.tensor_tensor(rv, ys_b, nuv, ALU.max)

        def emit_reduce(c):
            r_c = r_tiles.pop(c)
            for n2 in range(4):
                for b in range(4):
                    nc.tensor.matmul(
                        t_acc[32 * b:32 * (b + 1), 512 * n2:512 * (n2 + 1)],
                        et_all[:, 128 * c + 32 * b:128 * c + 32 * (b + 1)],
                        r_c[:, 2048 * b + 512 * n2:2048 * b + 512 * (n2 + 1)],
                        start=(c == 0),
                        stop=(c == NCHUNK - 1),
                        skip_group_check=True,
                        tile_position=(0, 32 * b),
                    )

        for cc in range(NCHUNK + SKEW):
            if cc < NCHUNK:
                emit_build(cc)
            if cc == 3:
                # mid-DVE-queue: rsum_ps is long done by now
                nc.vector.tensor_reduce(s_t[:], rsum_ps[:], AX.X, ALU.add)
                nc.vector.reciprocal(r_t[:], s_t[:])
                if mixed:
                    nc.vector.scalar_tensor_tensor(
                        junk8[:], rsum_ps[:], 0.0, maskM[:],
                        ALU.bypass, ALU.mult, accum_out=sM_t[:])
                    nc.vector.tensor_tensor(qcol[:], sM_t[:], r_t[:], ALU.mult)
            if cc >= SKEW:
                emit_reduce(cc - SKEW)

        if mixed:
            # scaled fix-up tile: nurep2q = nurep2 * (sM/s)  (ACT slack)
            fix_t = cons.tile([128, 2048], bf16)
            nc.scalar.activation(fix_t[:], nurep2[:], AF.Copy,
                                 bias=0.0, scale=qcol[:])
        else:
            fix_t = nurep2

        # ---- tail: o = t_acc*(1/s) - fix, fused on DVE; DMA out ----
        t_sb = cons.tile([128, 2048], f32)
        for n2 in range(4):
            sl = slice(512 * n2, 512 * (n2 + 1))
            nc.vector.scalar_tensor_tensor(
                t_sb[:, sl], t_acc[:, sl], r_t[:], fix_t[:, sl],
                ALU.mult, ALU.subtract)
            eng = nc.sync if n2 % 2 == 0 else nc.scalar
            eng.dma_start(o_d[:, sl], t_sb[:, sl])

    nc.compile()
    return nc


def _prep_inputs(x, adj, Wf, bf_, Ww, bw):
    b = ml_dtypes.bfloat16
    xT = np.ascontiguousarray(x.T)                        # [64, N]
    wfsT = np.ascontiguousarray(Wf[:, :DI].T)             # [64, 64]
    ws = Ww[0, :DI].reshape(DI, 1)                        # [64, 1]
    xm = np.hstack([xT, wfsT, ws]).astype(b)              # [64, 1089]
    nwfta = -np.vstack([Wf[:, DI:].T, bf_[None, :]])      # [65, 64]

    in_maps = []
    for c in range(N_CORES):
        blk = slice(ROWS * c, ROWS * (c + 1))
        xbTa = np.vstack([x[blk].T, np.ones((1, ROWS), np.float32)])
        xw = np.hstack([xbTa, nwfta]).astype(b)           # [65, 192]
        # adjT chunk-major: adjT[j_loc, 128c + i] = adj[blk0+i, 128c+j_loc]
        adjT = (adj[blk].T.reshape(NCHUNK, 128, ROWS)
                .transpose(1, 0, 2).reshape(128, N))
        in_maps.append(dict(xm=xm, xw=xw,
                            adjT=np.ascontiguousarray(adjT).astype(b)))
    return in_maps


def get_program():
    if "nc" not in _CACHE:
        _CACHE["nc"] = _build_program()
    return _CACHE["nc"]


def kernel(x, adj, Wf, bf, Ww, bw):
    x = np.asarray(x, dtype=np.float32)
    adj = np.asarray(adj, dtype=np.int32)
    Wf = np.asarray(Wf, dtype=np.float32)
    bf_ = np.asarray(bf, dtype=np.float32)
    Ww = np.asarray(Ww, dtype=np.float32)
    bw = np.asarray(bw, dtype=np.float32)
    assert x.shape == (N, DI) and adj.shape == (N, N)

    nc = get_program()
    in_maps = _prep_inputs(x, adj, Wf, bf_, Ww, bw)
    res = run_bass_kernel_spmd(nc, in_maps, core_ids=list(range(N_CORES)))
    p_idx = np.arange(128)
    col0 = (p_idx % 32) * DO
    out = np.empty((N, DO), np.float32)
    for c in range(N_CORES):
        t = res.results[c]["o"]                      # [128, 2048]
        out[ROWS * c:ROWS * (c + 1)] = t[p_idx[:, None],
                                         col0[:, None] + np.arange(DO)[None, :]]
    return out
